# revision 1
# baseline (speedup 1.0000x reference)
"""Trainium2 Bass kernel for nn_Codec (5-level lifting wavelet codec stats).

kernel(**inputs) takes the FULL inputs (x [32,3,512,512] f32 + eight 3-tap
filters) and returns (loss1, loss0, invCR0, invCR1) as np.float32 scalars.

Sharding: pure data parallel — 96 (batch*channel) slices split 12 per core
across 8 NeuronCores; scalar partials are gathered and reduced on the host.

Per-slice device pipeline:
  - 5-level lifting transform: x-phase convs (along W, the free dim) as
    VectorE scalar_tensor_tensor chains; y-phase (along H, partitions) as
    TensorE matmuls against host-composed banded matrices A_l (yo2) / B_l
    (ye2) built from the runtime taps.
  - Subbands packed into a [128, 2048] staging tile (exact tetris, the deep
    sub-128-partition bands moved by SBUF-to-SBUF DMA).
  - Wrapped residuals, binning, and radix-16 one-hot masks (bf16, DVE 4x);
    joint (h,l) counts accumulated on TensorE into PSUM, dumped raw to DRAM.
  - Sum-of-squares partials via ScalarE Square+accum / DVE fused reduces.
"""

import os

import numpy as np
from contextlib import ExitStack

import concourse.bass as bass
import concourse.mybir as mybir
import concourse.tile as tile
from concourse import bacc
from concourse.bass_utils import run_bass_kernel_spmd

F32 = mybir.dt.float32
BF16 = mybir.dt.bfloat16
I32 = mybir.dt.int32
ALU = mybir.AluOpType
ACTF = mybir.ActivationFunctionType

N_CORES = 8
S0 = 512
NSL = 12            # slices per core (96 / 8)
STG = 2048          # staging free dim per slice (512*512/128)
RES = S0 * S0
FC = 512            # mask chunk width (free dim)
N_LEVELS = 5

# tap vector layout in the "tp" dram tensor (broadcast to [128, NT] on chip).
# The on-chip DVE y-phase needs uy, ry and negated py, cy.
TP_UY, TP_RY, TP_NPY, TP_NCY = 0, 3, 6, 9
NT = 12

# ---------------------------------------------------------------------------
# host-side y-phase matrix composition
# ---------------------------------------------------------------------------


def _make_x_mats(S, px, ux, cx, rx):
    """A (xo2T = A@curT) and B (xe2T = B@curT), composed in float64, cast f32.
    The x-lifting always runs all four steps at every level."""
    half = S // 2
    E = np.zeros((half, S))
    O = np.zeros((half, S))
    E[np.arange(half), 2 * np.arange(half)] = 1.0
    O[np.arange(half), 2 * np.arange(half) + 1] = 1.0

    def T(k):
        M = np.zeros((half, half))
        i = np.arange(half)
        M[i, i] = k[1]
        M[i[1:], i[1:] - 1] = k[0]
        M[i[:-1], i[:-1] + 1] = k[2]
        return M

    Xo1 = O - T(px.astype(np.float64)) @ E
    Xe1 = E + T(ux.astype(np.float64)) @ Xo1
    A = Xo1 - T(cx.astype(np.float64)) @ Xe1
    B = Xe1 + T(rx.astype(np.float64)) @ A
    return A.astype(np.float32), B.astype(np.float32)


def _y_block_structure():
    """Static nonzero-block structure of A/B per level: for each out-tile r,
    the in-tile col indices c whose [128,128] block is structurally nonzero
    (composed band halfwidth in the S domain is <= 7)."""
    plans = []
    for lvl in range(N_LEVELS):
        S = S0 >> lvl
        half = S // 2
        t_out = max(1, half // 128)
        t_in = max(1, S // 128)
        rows = []
        for r in range(t_out):
            m0 = 128 * r
            m1 = min(m0 + 128, half)
            j0 = max(0, 2 * m0 - 10)
            j1 = min(S - 1, 2 * (m1 - 1) + 10)
            cs = [c for c in range(t_in) if (128 * c <= j1 and 128 * c + 127 >= j0)]
            rows.append(cs)
        plans.append(rows)  # same structure for A and B
    return plans


_Y_PLANS = _y_block_structure()
NW = 2 * sum(len(cs) for plan in _Y_PLANS for cs in plan)


def _build_wx_host(px, ux, cx, rx):
    """Pack transposed [K, M] blocks of A/B into wx [NW, 128, 128] f32, in
    the exact emission order of the device builder."""
    wy = np.zeros((NW, 128, 128), np.float32)
    i = 0
    for lvl in range(N_LEVELS):
        S = S0 >> lvl
        half = S // 2
        A, B = _make_x_mats(S, px, ux, cx, rx)
        for M_ in (A, B):
            for r, cs in enumerate(_Y_PLANS[lvl]):
                m0 = 128 * r
                m1 = min(m0 + 128, half)
                for c in cs:
                    k0 = 128 * c
                    k1 = min(k0 + 128, S)
                    wy[i, : k1 - k0, : m1 - m0] = M_[m0:m1, k0:k1].T
                    i += 1
    assert i == NW, (i, NW)
    return wy


def _verify_block_coverage(px, ux, cx, rx):
    # every nonzero of A/B must land in a packed block
    for lvl in range(N_LEVELS):
        S = S0 >> lvl
        half = S // 2
        A, B = _make_x_mats(S, px, ux, cx, rx)
        for M_ in (A, B):
            mass = np.abs(M_).sum()
            cov = 0.0
            for r, cs in enumerate(_Y_PLANS[lvl]):
                m0, m1 = 128 * r, min(128 * r + 128, half)
                for c in cs:
                    k0, k1 = 128 * c, min(128 * c + 128, S)
                    cov += np.abs(M_[m0:m1, k0:k1]).sum()
            assert abs(cov - mass) < 1e-6 * max(mass, 1), (lvl, cov, mass)


# staging slots (transposed orientation) for the deep subbands: exact tetris
# of the final 128 columns. (p0, p1, c0, c1)
DEEP_SLOTS = {
    "l2xo2": (0, 64, 1920, 2048),
    "l2yo2": (64, 128, 1920, 1984),
    "l3xo2": (64, 96, 1984, 2048),
    "l3yo2": (96, 128, 1984, 2016),
    "l4xo2": (96, 112, 2016, 2048),
    "l4yo2": (112, 128, 2016, 2032),
    "ye4": (112, 128, 2032, 2048),
}

# ---------------------------------------------------------------------------
# device kernel
# ---------------------------------------------------------------------------


def _conv_step(nc, out_ap, base_ap, src_ap, tap_col, tp_sb, P, F):
    """out = base + sum_taps: center, then left, then right tap (tap signs are
    folded into the tp columns). Free-dim conv with zero padding."""
    k0 = tp_sb[0:P, tap_col : tap_col + 1]
    k1 = tp_sb[0:P, tap_col + 1 : tap_col + 2]
    k2 = tp_sb[0:P, tap_col + 2 : tap_col + 3]
    nc.vector.scalar_tensor_tensor(out_ap, src_ap, k1, base_ap, ALU.mult, ALU.add)
    nc.vector.scalar_tensor_tensor(
        out_ap[:, 1:F], src_ap[:, 0 : F - 1], k0, out_ap[:, 1:F], ALU.mult, ALU.add
    )
    nc.vector.scalar_tensor_tensor(
        out_ap[:, 0 : F - 1], src_ap[:, 1:F], k2, out_ap[:, 0 : F - 1], ALU.mult, ALU.add
    )


def _hist_pipeline(nc, ctx, pools, src_ap_full, kind, psum_ap, acc, acc_base):
    """Wrap/bin/mask/matmul pipeline over a [128, STG] f32 source.

    kind "delta": floor-mod wrap with invalid (v < -1) exclusion; acc slots
    acc_base + {0: sumsq(dm1), 1: s2, 2: n0}.
    kind "img": x in [0,1); acc slot acc_base + 0 = sumsq.

    floor(x) is built as rne(x) - (rne(x) > x) since the ISA only has
    round-to-nearest-even f32->int conversion (no mod/divide/trunc).
    Scratch tags A..K are reused across disjoint lifetimes to fit SBUF.
    """
    sc = pools["scratch"]
    mpool = pools["masks"]
    bias_m1 = pools["bias_m1"]
    bias_128 = pools["bias_128"]

    if kind == "delta":
        # g = fl(fl(v+1) * 0.5)  (= u/2 exactly, u := fl(v+1))
        g = sc.tile([128, STG], F32, tag="A")
        nc.vector.tensor_scalar(g[:], src_ap_full, 1.0, 0.5, ALU.add, ALU.mult)
        gi = sc.tile([128, STG], I32, tag="B")
        nc.scalar.activation(gi[:], g[:], ACTF.Identity)  # rne convert on ACT
        cg = sc.tile([128, STG], F32, tag="C")
        nc.vector.tensor_tensor(cg[:], gi[:], g[:], ALU.is_gt)
        ff = sc.tile([128, STG], F32, tag="D")
        nc.vector.tensor_tensor(ff[:], gi[:], cg[:], ALU.subtract)  # floor(g)
        frac = sc.tile([128, STG], F32, tag="E")
        nc.vector.tensor_tensor(frac[:], g[:], ff[:], ALU.subtract)  # in [0,1)
        cneg = sc.tile([128, STG], F32, tag="G")
        nc.vector.tensor_scalar(cneg[:], src_ap_full, -1.0, None, ALU.is_lt)
        # sumsq accumulates Square(2*frac - 1) directly (dm1 = fl(2*frac-1))
        junk_act = sc.tile([128, STG], F32, tag="A")
        nc.scalar.activation(
            junk_act[:], frac[:], ACTF.Square, bias=bias_m1[:, 0:1], scale=2.0,
            accum_out=acc[:, acc_base : acc_base + 1],
        )
        # t = 256*frac (vs ((dm1+1)*128): sub-ulp path difference can move a
        # borderline element one bin; ~1e-7 effect)
        t = sc.tile([128, STG], F32, tag="B")
        nc.scalar.activation(t[:], frac[:], ACTF.Identity, scale=256.0)
        tb = sc.tile([128, STG], F32, tag="C")
        nc.vector.scalar_tensor_tensor(tb[:], cneg[:], -512.0, t[:], ALU.mult, ALU.add)
        # s2 = sum over invalid of (8 - 4m) = (frac * -8 + 8) * cneg
        junk_dve = sc.tile([128, STG], F32, tag="B")
        nc.vector.affine_mul_reduce(
            junk_dve[:], acc[:, acc_base + 1 : acc_base + 2], frac[:], cneg[:], -8.0, 8.0
        )
        tsrc = tb
        n_h, h0, G = 16, 0, 8
    else:
        dm1 = sc.tile([128, STG], F32, tag="F")
        nc.vector.tensor_scalar(dm1[:], src_ap_full, 1.0, 1.0, ALU.add, ALU.subtract)
        junk_act = sc.tile([128, STG], F32, tag="A")
        nc.scalar.activation(
            junk_act[:], dm1[:], ACTF.Square, accum_out=acc[:, acc_base : acc_base + 1]
        )
        t = sc.tile([128, STG], F32, tag="B")
        nc.scalar.activation(t[:], dm1[:], ACTF.Identity, bias=bias_128[:, 0:1], scale=128.0)
        tsrc = t
        n_h, h0, G = 8, 8, 16

    # binf = floor(tsrc) via rne(tsrc - (0.5 - 2^-17)): the epsilon breaks
    # rne ties on exact-integer t (exact for img bins; for delta a sub-ulp
    # zone can shift ~2 counts/slice to an adjacent bin, ~1e-7 on entropy)
    bi2 = sc.tile([128, STG], I32, tag="E")
    nc.vector.tensor_scalar(bi2[:], tsrc[:], -0.49999237060546875, None, ALU.add)
    l_i = sc.tile([128, STG], I32, tag="F")
    nc.vector.tensor_scalar(l_i[:], bi2[:], 15, None, ALU.bitwise_and)
    h_i = sc.tile([128, STG], I32, tag="G")
    nc.vector.tensor_scalar(h_i[:], bi2[:], 4, None, ALU.arith_shift_right)
    lb = sc.tile([128, STG], BF16, tag="J")
    nc.scalar.activation(lb[:], l_i[:], ACTF.Identity)
    hb = sc.tile([128, STG], BF16, tag="K")
    nc.scalar.activation(hb[:], h_i[:], ACTF.Identity)

    n_chunks = STG // FC
    n_mm = FC // G
    for ch in range(n_chunks):
        c0 = ch * FC
        # layout [128, n_mm, bins*G]: sub-chunk g's operand is contiguous
        # (walrus requires a single free dim on matmul operands)
        mh = mpool.tile([128, n_mm, n_h * G], BF16, tag="mh")
        ml = mpool.tile([128, n_mm, 16 * G], BF16, tag="ml")
        if ABL["masks"]:
            for a in range(n_h):
                nc.vector.tensor_scalar(
                    mh[:, :, a * G : (a + 1) * G],
                    hb[:, c0 : c0 + FC],
                    float(h0 + a),
                    None,
                    ALU.is_equal,
                )
            for b in range(16):
                nc.vector.tensor_scalar(
                    ml[:, :, b * G : (b + 1) * G],
                    lb[:, c0 : c0 + FC],
                    float(b),
                    None,
                    ALU.is_equal,
                )
        else:
            nc.vector.memset(mh[:], 0.0)
            nc.vector.memset(ml[:], 0.0)
        if ABL["mm"]:
            for g_ in range(n_mm):
                nc.tensor.matmul(
                    psum_ap,
                    mh[:, g_, :],
                    ml[:, g_, :],
                    start=(ch == 0 and g_ == 0),
                    stop=(ch == n_chunks - 1 and g_ == n_mm - 1),
                    skip_group_check=True,
                )
        elif ch == 0:
            nc.tensor.matmul(
                psum_ap, mh[:, 0, :], ml[:, 0, :], start=True, stop=True,
                skip_group_check=True,
            )


ABL = {"masks": True, "mm": True, "prep": True, "x": True, "y": True, "hist_d": True, "hist_i": True}


def build_nc(nsl=NSL):
    nc = bacc.Bacc("TRN2", target_bir_lowering=False, debug=False)
    xs = nc.dram_tensor("xs", [nsl, S0, S0], F32, kind="ExternalInput")
    tp = nc.dram_tensor("tp", [NT], F32, kind="ExternalInput")
    wy = nc.dram_tensor("wy", [NW, 128, 128], F32, kind="ExternalInput")
    pd = nc.dram_tensor("pd", [nsl, 128, 128], F32, kind="ExternalOutput")
    pi = nc.dram_tensor("pi", [nsl, 128, 256], F32, kind="ExternalOutput")
    accd = nc.dram_tensor("accd", [128, nsl * 8], F32, kind="ExternalOutput")

    with tile.TileContext(nc) as tc:
        with ExitStack() as ctx:
            const = ctx.enter_context(tc.tile_pool(name="const", bufs=1))
            xpool = ctx.enter_context(tc.tile_pool(name="xpool", bufs=2))
            stgp = ctx.enter_context(tc.tile_pool(name="stgp", bufs=2))
            work = ctx.enter_context(tc.tile_pool(name="work", bufs=3))
            xe2p = ctx.enter_context(tc.tile_pool(name="xe2p", bufs=3))
            scratch = ctx.enter_context(tc.tile_pool(name="scratch", bufs=1))
            maskp = ctx.enter_context(tc.tile_pool(name="masks", bufs=2))
            psum = ctx.enter_context(tc.tile_pool(name="psum", bufs=2, space="PSUM"))
            ypsum = ctx.enter_context(tc.tile_pool(name="ypsum", bufs=4, space="PSUM"))
            bias_m1 = const.tile([128, 1], F32, tag="bias_m1")
            nc.vector.memset(bias_m1[:], -1.0)
            bias_128 = const.tile([128, 1], F32, tag="bias_128")
            nc.vector.memset(bias_128[:], 128.0)
            pools = {"scratch": scratch, "masks": maskp,
                     "bias_m1": bias_m1, "bias_128": bias_128}

            tp_sb = const.tile([128, NT], F32)
            nc.sync.dma_start(
                tp_sb[:], tp.ap().rearrange("(o n) -> o n", o=1).broadcast_to([128, NT])
            )
            wy_sb = const.tile([128, NW * 128], F32)
            nc.sync.dma_start(
                wy_sb[:].rearrange("k (n m) -> k n m", n=NW),
                wy.ap().rearrange("n k m -> k n m"),
            )
            acc = const.tile([128, nsl * 8], F32)
            nc.vector.memset(acc[:], 0.0)

            for s in range(nsl):
                # load slice: x_sb[p, t, w] = xs[s, 128t + p, w]
                x_sb = xpool.tile([128, 4, S0], F32, tag="x_sb")
                nc.sync.dma_start(
                    x_sb[:], xs.ap()[s].rearrange("(t p) w -> p t w", p=128)
                )
                x_flat = x_sb[:].rearrange("p t w -> p (t w)")

                stg = stgp.tile([128, STG], F32, tag="stg")

                cur_tiles = [x_sb[:, t, :] for t in range(4)]
                wy_idx = 0
                for lvl in range(N_LEVELS):
                    S = S0 >> lvl
                    half = S // 2
                    t_out = max(1, half // 128)

                    # ---- x-phase (PE): xo2T = A@curT (kind 0), xe2T = B@curT
                    xe2_tiles = []
                    for kind in range(2):
                        for r in range(t_out):
                            m0 = 128 * r
                            m1 = min(m0 + 128, half)
                            M = m1 - m0
                            cs = _Y_PLANS[lvl][r]
                            ps = ypsum.tile([M, S], F32, tag="yps")
                            for i, c in enumerate(cs):
                                K = min(128, S - 128 * c)
                                nc.tensor.matmul(
                                    ps[0:M, 0:S],
                                    wy_sb[0:K, 128 * wy_idx : 128 * wy_idx + M],
                                    cur_tiles[c][0:K, 0:S],
                                    start=(i == 0),
                                    stop=(i == len(cs) - 1),
                                    skip_group_check=True,
                                )
                                wy_idx += 1
                            if kind == 0:
                                if lvl == 0:
                                    nc.scalar.copy(
                                        stg[:, 512 * r : 512 * (r + 1)], ps[0:M, 0:S]
                                    )
                                elif lvl == 1:
                                    nc.scalar.copy(stg[:, 1536:1792], ps[0:M, 0:S])
                                elif lvl == 2:
                                    p0, p1, q0, q1 = DEEP_SLOTS["l2xo2"]
                                    nc.scalar.copy(stg[p0:p1, q0:q1], ps[0:M, 0:S])
                                else:
                                    key = "l3xo2" if lvl == 3 else "l4xo2"
                                    p0, p1, q0, q1 = DEEP_SLOTS[key]
                                    xo2s = work.tile([M, S], F32, tag=f"xo2s_{lvl}")
                                    nc.scalar.copy(xo2s[:], ps[0:M, 0:S])
                                    nc.sync.dma_start(stg[p0:p1, q0:q1], xo2s[:])
                            else:
                                xe2 = xe2p.tile([M, S], F32, tag=f"xe2_{lvl}")
                                nc.scalar.copy(xe2[:], ps[0:M, 0:S])
                                xe2_tiles.append((xe2, M))

                    # ---- y-phase (DVE) per xe2 tile
                    new_cur = []
                    for ti, (xe2, P) in enumerate(xe2_tiles):
                        ye_v = xe2[0:P, 0:S:2]
                        yo_v = xe2[0:P, 1:S:2]
                        yo1 = work.tile([P, half], F32, tag=f"yo1_{lvl}")
                        _conv_step(nc, yo1[:], yo_v, ye_v, TP_NPY, tp_sb, P, half)
                        if lvl < 2:
                            ye1 = work.tile([P, half], F32, tag=f"ye1_{lvl}")
                            _conv_step(nc, ye1[:], ye_v, yo1[:], TP_UY, tp_sb, P, half)
                            ye1_ap = ye1[:]
                        else:
                            ye1_ap = ye_v
                        if lvl == 0:
                            yo2 = stg[:, 1024 + 256 * ti : 1024 + 256 * (ti + 1)]
                        elif lvl == 1:
                            yo2 = stg[:, 1792:1920]
                        else:
                            yo2_t = work.tile([P, half], F32, tag=f"yo2_{lvl}")
                            yo2 = yo2_t[:]
                        _conv_step(nc, yo2, yo1[:], ye1_ap, TP_NCY, tp_sb, P, half)
                        if lvl >= 2:
                            key = {2: "l2yo2", 3: "l3yo2", 4: "l4yo2"}[lvl]
                            p0, p1, q0, q1 = DEEP_SLOTS[key]
                            nc.sync.dma_start(stg[p0:p1, q0:q1], yo2)
                        ye2 = work.tile([P, half], F32, tag=f"ye2_{lvl}")
                        _conv_step(nc, ye2[:], ye1_ap, yo2, TP_RY, tp_sb, P, half)
                        if lvl < N_LEVELS - 1:
                            new_cur.append(ye2[:])
                        else:
                            p0, p1, q0, q1 = DEEP_SLOTS["ye4"]
                            nc.sync.dma_start(stg[p0:p1, q0:q1], ye2[:])
                    cur_tiles = new_cur

                # ---- histograms + stats (img first: it only needs x_sb, so
                # its DVE mask work can fill the lifting ladder's bubbles)
                if ABL["hist_i"]:
                    ps_i = psum.tile([128, 256], F32, tag="ps_i")
                    _hist_pipeline(nc, ctx, pools, x_flat, "img", ps_i[:], acc, s * 8 + 4)
                    pi_sb = work.tile([128, 256], F32, tag="pi_sb")
                    nc.scalar.copy(pi_sb[:], ps_i[:])
                    nc.sync.dma_start(pi.ap()[s], pi_sb[:])

                if ABL["hist_d"]:
                    ps_d = psum.tile([128, 128], F32, tag="ps_d")
                    _hist_pipeline(nc, ctx, pools, stg[:], "delta", ps_d[:], acc, s * 8)
                    pd_sb = work.tile([128, 128], F32, tag="pd_sb")
                    nc.scalar.copy(pd_sb[:], ps_d[:])
                    nc.sync.dma_start(pd.ap()[s], pd_sb[:])

            nc.sync.dma_start(accd.ap()[:, :], acc[:])

    nc.compile()
    return nc


_NC_CACHE = {}


def _get_nc():
    if "nc" not in _NC_CACHE:
        _NC_CACHE["nc"] = build_nc()
    return _NC_CACHE["nc"]


LAST_INFO = {}


def kernel(x, px, ux, cx, rx, py, uy, cy, ry, _trace=False):
    x = np.ascontiguousarray(np.asarray(x, dtype=np.float32))
    px, ux, cx, rx, py, uy, cy, ry = (
        np.asarray(k, dtype=np.float32) for k in (px, ux, cx, rx, py, uy, cy, ry)
    )

    nc = _get_nc()

    tp_host = np.zeros(NT, np.float32)
    tp_host[TP_UY : TP_UY + 3] = uy
    tp_host[TP_RY : TP_RY + 3] = ry
    tp_host[TP_NPY : TP_NPY + 3] = -py
    tp_host[TP_NCY : TP_NCY + 3] = -cy
    wy_host = _build_wx_host(px, ux, cx, rx)

    # the device works on W-major (transposed) slices so the x-phase convs
    # run along partitions (TensorE) and the y-phase along the free dim
    shards = np.ascontiguousarray(
        x.reshape(N_CORES, NSL, S0, S0).transpose(0, 1, 3, 2)
    )
    in_maps = [
        {"xs": np.ascontiguousarray(shards[i]), "tp": tp_host, "wy": wy_host}
        for i in range(N_CORES)
    ]
    if not _trace:
        # the axon trace path needs antenv.axon_hooks, which this container
        # lacks; make sure an inherited BASS_TRACE can't route us there
        os.environ.setdefault("BASS_NEVER_TRACE", "1")
    res = run_bass_kernel_spmd(nc, in_maps, core_ids=list(range(N_CORES)), trace=_trace)
    LAST_INFO["exec_time_ns"] = res.exec_time_ns
    LAST_INFO["results"] = res

    counts_img = np.zeros((96, 256))
    counts_delta = np.zeros((96, 256))
    ss_img = np.zeros(96)
    ss_delta = np.zeros(96)
    for core in range(N_CORES):
        out = res.results[core]
        pd_ = out["pd"].astype(np.float64)
        pi_ = out["pi"].astype(np.float64)
        acc_ = out["accd"].astype(np.float64).sum(axis=0)
        for s in range(NSL):
            gs = core * NSL + s
            cd = np.einsum("afbf->ab", pd_[s].reshape(16, 8, 16, 8)).reshape(256)
            ci = np.einsum("afbf->ab", pi_[s].reshape(8, 16, 16, 16)).reshape(128)
            a = acc_[s * 8 : s * 8 + 8]
            sumsq_d, s2, n0, sumsq_i = a[0], a[1], a[2], a[4]
            cd[0] += n0
            counts_delta[gs] = cd
            counts_img[gs, 128:256] = ci
            ss_delta[gs] = sumsq_d + s2 - 8.0 * n0
            ss_img[gs] = sumsq_i

    loss1 = np.float32(255.0 * np.sqrt(ss_delta.sum() / (96 * RES)))
    loss0 = np.float32(255.0 * np.sqrt(ss_img.sum() / (96 * RES)))

    def ent(counts):
        p = counts / RES
        pz = np.where(p > 0, p, 1.0)
        return float(np.sum(-p * np.log2(pz)))

    invCR0 = np.float32(ent(counts_img) / (8.0 * 96))
    invCR1 = np.float32(ent(counts_delta) / (8.0 * 96))
    LAST_INFO.update(
        counts_img=counts_img, counts_delta=counts_delta, ss_img=ss_img, ss_delta=ss_delta
    )
    return loss1, loss0, invCR0, invCR1



# revision 7
# speedup vs baseline: 1.2845x; 1.2845x over previous
"""Trainium2 Bass kernel for nn_Codec (5-level lifting wavelet codec stats).

kernel(**inputs) takes the FULL inputs (x [32,3,512,512] f32 + eight 3-tap
filters) and returns (loss1, loss0, invCR0, invCR1) as np.float32 scalars.

Sharding: pure data parallel - 96 (batch*channel) slices split 12 per core
across 8 NeuronCores; scalar partials are gathered and reduced on the host.

Per-slice device pipeline (v3):
  - Lifting levels 0-1 run entirely on TensorE as banded matmuls against
    host-composed matrices: x-phase (along W = partitions) with fp32r (lvl0)
    / bf16 (lvl1) stationaries, then a bf16 DMA-xbar transpose and the
    y-phase as matmuls too. Levels 2-4 (tiny) keep x-phase matmuls and
    DVE scalar_tensor_tensor y-convs.
  - Subbands land in a [128, 2048] f32 staging tile (any orientation - the
    histogram is orientation-agnostic); deep sub-128-partition bands are
    placed by small SBUF-to-SBUF DMAs.
  - Integer-direct binning: bi = rne(v*128 + 127.5+2^-10) (== floor of the
    scaled value for dyadic inputs), bin = bi & 255, digits by shift/mask;
    invalid (v < -1) elements are pushed out of the l one-hot range.
  - Radix-16x16 (delta) and 16x8 (img) one-hot bf16 masks (DVE 4x mode,
    partially offloaded to GPSIMD), joint counts via TensorE into PSUM.
  - RMSE sums via ScalarE Square+accum; the fmod-vs-posmod correction via a
    fused affine_mul_reduce.
"""

import os

import numpy as np
from contextlib import ExitStack

import concourse.bass as bass
import concourse.mybir as mybir
import concourse.tile as tile
from concourse import bacc
from concourse.bass_utils import run_bass_kernel_spmd

F32 = mybir.dt.float32
F32R = mybir.dt.float32r
BF16 = mybir.dt.bfloat16
I32 = mybir.dt.int32
ALU = mybir.AluOpType
ACTF = mybir.ActivationFunctionType

N_CORES = 8
S0 = 512
NSL = 12            # slices per core (96 / 8)
STG = 2048          # staging free dim per slice (512*512/128)
RES = S0 * S0
FC = 512            # mask chunk width (free dim)
N_LEVELS = 5
G = 8               # sub-chunk dup factor for the joint-count matmuls
C0 = 127.5009765625  # 127.5 + 2^-10: rne(v*128 + C0) == 128 + floor(v*128)

# tap vector layout (y-phase DVE convs for levels 2-4 only).
TP_RY, TP_NPY, TP_NCY = 0, 3, 6
NT = 9

# ---------------------------------------------------------------------------
# host-side matrix composition + block plans
# ---------------------------------------------------------------------------


def _make_mats(S, p, u, c, r, update):
    """A (odd out) and B (even out) lifting matrices [S/2, S], f64->f32.
    update=False omits the update step (y-lifting at levels >= 2)."""
    half = S // 2
    E = np.zeros((half, S))
    O = np.zeros((half, S))
    E[np.arange(half), 2 * np.arange(half)] = 1.0
    O[np.arange(half), 2 * np.arange(half) + 1] = 1.0

    def T(k):
        M = np.zeros((half, half))
        i = np.arange(half)
        M[i, i] = k[1]
        M[i[1:], i[1:] - 1] = k[0]
        M[i[:-1], i[:-1] + 1] = k[2]
        return M

    o1 = O - T(p.astype(np.float64)) @ E
    e1 = E + T(u.astype(np.float64)) @ o1 if update else E
    A = o1 - T(c.astype(np.float64)) @ e1
    B = e1 + T(r.astype(np.float64)) @ A
    return A.astype(np.float32), B.astype(np.float32)


def _plan(S):
    """Static nonzero-block structure for a [S/2, S] composed lifting matrix
    (band halfwidth <= 10 in the S domain): per out-tile r, the list of
    128-wide K-block cols that are structurally nonzero."""
    half = S // 2
    t_out = max(1, (half + 127) // 128)
    t_in = max(1, (S + 127) // 128)
    rows = []
    for r in range(t_out):
        m0 = 128 * r
        m1 = min(m0 + 128, half)
        j0 = max(0, 2 * m0 - 10)
        j1 = min(S - 1, 2 * (m1 - 1) + 10)
        rows.append([c for c in range(t_in) if 128 * c <= j1 and 128 * c + 127 >= j0])
    return rows


X_PLANS = [_plan(S0 >> lvl) for lvl in range(N_LEVELS)]
Y_PLANS = [_plan(S0 >> lvl) for lvl in range(2)]  # y on PE for lvl 0-1 only

# block counts: wyf (f32): lvl3/lvl4 x A/B; wyb (bf16): lvl0 x A/B + lvl0 y
# C/R + lvl1 x A/B + lvl1 y C/R + lvl2 x A/B
NWF = 2 * len(X_PLANS[3][0]) * 1 * 2
NWB = (
    2 * sum(len(cs) for cs in X_PLANS[0])
    + 2 * sum(len(cs) for cs in Y_PLANS[0])
    + 2 * sum(len(cs) for cs in X_PLANS[1])
    + 2 * sum(len(cs) for cs in Y_PLANS[1])
    + 2 * sum(len(cs) for cs in X_PLANS[2])
)


def _pack_blocks(M_, plan, buf, i):
    half, S = M_.shape[0], M_.shape[1]
    for r, cs in enumerate(plan):
        m0, m1 = 128 * r, min(128 * r + 128, half)
        for c in cs:
            k0, k1 = 128 * c, min(128 * c + 128, S)
            buf[i, : k1 - k0, : m1 - m0] = M_[m0:m1, k0:k1].T
            i += 1
    return i


def _build_w_host(px, ux, cx, rx, py, uy, cy, ry):
    """Pack transposed [K, M] blocks into wyf [NWF,128,128] f32 and
    wyb [NWB,128,128] f32 (cast to bf16 by the caller), in the exact
    emission order of the device builder. Also verifies band coverage."""
    xmats = [_make_mats(S0 >> l, px, ux, cx, rx, update=True) for l in range(N_LEVELS)]
    ymats = [_make_mats(S0 >> l, py, uy, cy, ry, update=(l < 2)) for l in range(2)]

    for lvl in range(N_LEVELS):
        for M_ in xmats[lvl]:
            _check_cover(M_, X_PLANS[lvl])
    for lvl in range(2):
        for M_ in ymats[lvl]:
            _check_cover(M_, Y_PLANS[lvl])

    wyf = np.zeros((NWF, 128, 128), np.float32)
    i = 0
    for lvl in (3, 4):
        for M_ in xmats[lvl]:
            i = _pack_blocks(M_, X_PLANS[lvl], wyf, i)
    assert i == NWF, (i, NWF)

    wyb = np.zeros((NWB, 128, 128), np.float32)
    i = 0
    for M_ in xmats[0]:
        i = _pack_blocks(M_, X_PLANS[0], wyb, i)
    for M_ in ymats[0]:
        i = _pack_blocks(M_, Y_PLANS[0], wyb, i)
    for M_ in xmats[1]:
        i = _pack_blocks(M_, X_PLANS[1], wyb, i)
    for M_ in ymats[1]:
        i = _pack_blocks(M_, Y_PLANS[1], wyb, i)
    for M_ in xmats[2]:
        i = _pack_blocks(M_, X_PLANS[2], wyb, i)
    assert i == NWB, (i, NWB)
    return wyf, wyb


def _check_cover(M_, plan):
    half, S = M_.shape
    mass = np.abs(M_).sum()
    cov = 0.0
    for r, cs in enumerate(plan):
        m0, m1 = 128 * r, min(128 * r + 128, half)
        for c in cs:
            k0, k1 = 128 * c, min(128 * c + 128, S)
            cov += np.abs(M_[m0:m1, k0:k1]).sum()
    assert abs(cov - mass) < 1e-6 * max(mass, 1), (half, S, cov, mass)


# staging slots for the deep subbands (levels 2-4): exact tetris of the final
# 128 columns. (p0, p1, c0, c1)
DEEP_SLOTS = {
    "l2xo2": (0, 64, 1920, 2048),
    "l2yo2": (64, 128, 1920, 1984),
    "l3xo2": (64, 96, 1984, 2048),
    "l3yo2": (96, 128, 1984, 2016),
    "l4xo2": (96, 112, 2016, 2048),
    "l4yo2": (112, 128, 2016, 2032),
    "ye4": (112, 128, 2032, 2048),
}

# engine routing for elementwise prep ops: "v" = DVE, "g" = GPSIMD
ROUTE = {}
# number of mask planes (per chunk) routed to GPSIMD, taken from the front of
# the delta h-plane list
POOL_PLANES_DH = 0
POOL_PLANES_IH = 0

# ---------------------------------------------------------------------------
# device kernel
# ---------------------------------------------------------------------------


def _conv_step(nc, out_ap, base_ap, src_ap, tap_col, tp_sb, P, F):
    """out = base + 3-tap conv of src along the free dim, zero padding."""
    k0 = tp_sb[0:P, tap_col : tap_col + 1]
    k1 = tp_sb[0:P, tap_col + 1 : tap_col + 2]
    k2 = tp_sb[0:P, tap_col + 2 : tap_col + 3]
    nc.vector.scalar_tensor_tensor(out_ap, src_ap, k1, base_ap, ALU.mult, ALU.add)
    nc.vector.scalar_tensor_tensor(
        out_ap[:, 1:F], src_ap[:, 0 : F - 1], k0, out_ap[:, 1:F], ALU.mult, ALU.add
    )
    nc.vector.scalar_tensor_tensor(
        out_ap[:, 0 : F - 1], src_ap[:, 1:F], k2, out_ap[:, 0 : F - 1], ALU.mult, ALU.add
    )


def _hist_pipeline(nc, eng, pools, hb_src, lb_src, n_h, h0, n_l, psum_ap, pool_h):
    """One-hot mask + joint-count matmuls over [128, STG] bf16 digit sources.
    hb_src/lb_src: bf16 digit tensors (l already exclusion-shifted for delta).
    n_h h-values starting at h0 (stationary side), n_l l-values (moving).
    pool_h: how many h-planes per chunk go to GPSIMD."""
    mpool = pools["masks"]
    n_chunks = STG // FC
    n_mm = FC // G
    for ch in range(n_chunks):
        c0 = ch * FC
        mh = mpool.tile([128, n_mm, 16 * G], BF16, tag="mh")
        ml = mpool.tile([128, n_mm, 16 * G], BF16, tag="ml")
        for a in range(n_h):
            e = nc.gpsimd if a < pool_h else nc.vector
            e.tensor_scalar(
                mh[:, :, a * G : (a + 1) * G],
                hb_src[:, c0 : c0 + FC],
                float(h0 + a),
                None,
                ALU.is_equal,
            )
        for b in range(n_l):
            nc.vector.tensor_scalar(
                ml[:, :, b * G : (b + 1) * G],
                lb_src[:, c0 : c0 + FC],
                float(b),
                None,
                ALU.is_equal,
            )
        for g_ in range(n_mm):
            nc.tensor.matmul(
                psum_ap,
                mh[:, g_, 0 : n_h * G],
                ml[:, g_, 0 : n_l * G],
                start=(ch == 0 and g_ == 0),
                stop=(ch == n_chunks - 1 and g_ == n_mm - 1),
                skip_group_check=True,
            )


def build_nc(nsl=NSL):
    nc = bacc.Bacc("TRN2", target_bir_lowering=False, debug=False)
    xs = nc.dram_tensor("xs", [nsl, S0, S0], F32, kind="ExternalInput")
    tp = nc.dram_tensor("tp", [NT], F32, kind="ExternalInput")
    wyf = nc.dram_tensor("wyf", [NWF, 128, 128], F32, kind="ExternalInput")
    wyb = nc.dram_tensor("wyb", [NWB, 128, 128], BF16, kind="ExternalInput")
    pd = nc.dram_tensor("pd", [nsl, 128, 128], F32, kind="ExternalOutput")
    pi = nc.dram_tensor("pi", [nsl, 128, 64], F32, kind="ExternalOutput")
    accd = nc.dram_tensor("accd", [128, nsl * 8], F32, kind="ExternalOutput")

    def V(name):
        return nc.gpsimd if ROUTE.get(name) == "g" else nc.vector

    with tile.TileContext(nc) as tc:
        with ExitStack() as ctx:
            const = ctx.enter_context(tc.tile_pool(name="const", bufs=1))
            xpool = ctx.enter_context(tc.tile_pool(name="xpool", bufs=2))
            stgp = ctx.enter_context(tc.tile_pool(name="stgp", bufs=2))
            lift = ctx.enter_context(tc.tile_pool(name="lift", bufs=2))
            work = ctx.enter_context(tc.tile_pool(name="work", bufs=3))
            scratch = ctx.enter_context(tc.tile_pool(name="scratch", bufs=1))
            maskp = ctx.enter_context(tc.tile_pool(name="masks", bufs=2))
            psum = ctx.enter_context(tc.tile_pool(name="psum", bufs=2, space="PSUM"))
            ypsum = ctx.enter_context(tc.tile_pool(name="ypsum", bufs=2, space="PSUM"))

            tp_sb = const.tile([128, NT], F32)
            nc.sync.dma_start(
                tp_sb[:], tp.ap().rearrange("(o n) -> o n", o=1).broadcast_to([128, NT])
            )
            wyf_sb = const.tile([128, NWF * 128], F32)
            nc.sync.dma_start(
                wyf_sb[:].rearrange("k (n m) -> k n m", n=NWF),
                wyf.ap().rearrange("n k m -> k n m"),
            )
            wyb_sb = const.tile([128, NWB * 128], BF16)
            nc.sync.dma_start(
                wyb_sb[:].rearrange("k (n m) -> k n m", n=NWB),
                wyb.ap().rearrange("n k m -> k n m"),
            )
            acc = const.tile([128, nsl * 8], F32)
            nc.vector.memset(acc[:], 0.0)
            pools = {"masks": maskp}

            # wyf/wyb block cursors are re-derived per slice; define the
            # per-matrix starting indices once.
            nb_x0 = 2 * sum(len(cs) for cs in X_PLANS[0])
            nb_y0 = nb_x0 + 2 * sum(len(cs) for cs in Y_PLANS[0])
            nb_x1 = nb_y0 + 2 * sum(len(cs) for cs in X_PLANS[1])
            nb_y1 = nb_x1 + 2 * sum(len(cs) for cs in Y_PLANS[1])

            for s in range(nsl):
                # ---- load slice (transposed): x_sb[p, t, h] = xs[s, 128t+p, h]
                x_sb = xpool.tile([128, 4, S0], F32, tag="x_sb")
                nc.sync.dma_start(
                    x_sb[:], xs.ap()[s].rearrange("(t p) w -> p t w", p=128)
                )
                x_flat = x_sb[:].rearrange("p t w -> p (t w)")

                stg = stgp.tile([128, STG], F32, tag="stg")

                # ================= level 0 (PE, bf16) =====================
                xb = lift.tile([128, 4, S0], BF16, tag="xb")
                nc.scalar.copy(
                    xb[:].rearrange("p t w -> p (t w)"), x_flat
                )
                xe2b = lift.tile([128, 2, S0], BF16, tag="xe2b0")
                wf = 0
                for kind in range(2):  # 0: A (xo), 1: B (xe2)
                    for r, cs in enumerate(X_PLANS[0]):
                        ps = ypsum.tile([128, 512], F32, tag="yps")
                        for i, c in enumerate(cs):
                            nc.tensor.matmul(
                                ps[:, :],
                                wyb_sb[0:128, 128 * wf : 128 * wf + 128],
                                xb[:, c, :],
                                start=(i == 0),
                                stop=(i == len(cs) - 1),
                                skip_group_check=True,
                            )
                            wf += 1
                        if kind == 0:
                            nc.scalar.copy(stg[:, 512 * r : 512 * (r + 1)], ps[:, :])
                        else:
                            nc.scalar.copy(xe2b[:, r, :], ps[:, :])
                assert wf == nb_x0

                # transpose xe2 [256, 512] -> xe2T [512, 256] (bf16 xbar)
                # xe2T[p, r, t, :]: h-block t, w2 in [128r, 128r+128)
                xe2T = lift.tile([128, 2, 4, 128], BF16, tag="xe2T0")
                for r in range(2):
                    nc.sync.dma_start_transpose(xe2T[:, r, :, :], xe2b[:, r, :])

                # y-phase: yo = C0 @ xe2T, ye2 = R0 @ xe2T, per (r2, r)
                ye2b = lift.tile([128, 2, 2, 128], BF16, tag="ye2b0")
                wb = nb_x0
                for kind in range(2):  # 0: C (yo), 1: R (ye2)
                    for r2, cs in enumerate(Y_PLANS[0]):
                        blk0 = wb
                        for r in range(2):
                            ps2f = ypsum.tile([128, 512], F32, tag="yps")
                            ps2 = ps2f[:, 0:128]
                            for i, c2 in enumerate(cs):
                                nc.tensor.matmul(
                                    ps2,
                                    wyb_sb[0:128, 128 * (blk0 + i) : 128 * (blk0 + i) + 128],
                                    xe2T[:, r, c2, :],
                                    start=(i == 0),
                                    stop=(i == len(cs) - 1),
                                    skip_group_check=True,
                                )
                            if kind == 0:
                                col = 1024 + 128 * (2 * r2 + r)
                                nc.scalar.copy(stg[:, col : col + 128], ps2)
                            else:
                                nc.scalar.copy(ye2b[:, r2, r, :], ps2)
                        wb = blk0 + len(cs)
                assert wb == nb_y0

                # cur1 = ye2^T: per block (r2, r) -> cur1[:, r, 128r2:...]
                cur1 = lift.tile([128, 2, 256], BF16, tag="cur1")
                for r2 in range(2):
                    for r in range(2):
                        nc.sync.dma_start_transpose(
                            cur1[:, r, 128 * r2 : 128 * r2 + 128], ye2b[:, r2, r, :]
                        )

                # ================= level 1 (PE bf16) ======================
                xe2b1 = lift.tile([128, 256], BF16, tag="xe2b1")
                for kind in range(2):
                    base = nb_y0 + (0 if kind == 0 else len(X_PLANS[1][0]))
                    psf = ypsum.tile([128, 512], F32, tag="yps")
                    ps = psf[:, 0:256]
                    cs = X_PLANS[1][0]
                    for i, c in enumerate(cs):
                        nc.tensor.matmul(
                            ps,
                            wyb_sb[0:128, 128 * (base + i) : 128 * (base + i) + 128],
                            cur1[:, c, :],
                            start=(i == 0),
                            stop=(i == len(cs) - 1),
                            skip_group_check=True,
                        )
                    if kind == 0:
                        nc.scalar.copy(stg[:, 1536:1792], ps)
                    else:
                        nc.scalar.copy(xe2b1[:], ps)

                xe2T1 = lift.tile([128, 2, 128], BF16, tag="xe2T1")
                nc.sync.dma_start_transpose(xe2T1[:, :, :], xe2b1[:])

                ye2b1 = lift.tile([128, 128], BF16, tag="ye2b1")
                for kind in range(2):
                    base = nb_x1 + (0 if kind == 0 else len(Y_PLANS[1][0]))
                    ps2f = ypsum.tile([128, 512], F32, tag="yps")
                    ps2 = ps2f[:, 0:128]
                    cs = Y_PLANS[1][0]
                    for i, c2 in enumerate(cs):
                        nc.tensor.matmul(
                            ps2,
                            wyb_sb[0:128, 128 * (base + i) : 128 * (base + i) + 128],
                            xe2T1[:, c2, :],
                            start=(i == 0),
                            stop=(i == len(cs) - 1),
                            skip_group_check=True,
                        )
                    if kind == 0:
                        nc.scalar.copy(stg[:, 1792:1920], ps2)
                    else:
                        nc.scalar.copy(ye2b1[:], ps2)

                cur2 = lift.tile([128, 128], BF16, tag="cur2")
                nc.sync.dma_start_transpose(cur2[:, :], ye2b1[:])

                # ================= levels 2-4 (PE x, DVE y) ===============
                cur_tiles = [(cur2[:, :], 128, True)]  # (ap, K, is_bf16)
                wf34 = 0
                wb2 = nb_y1
                for lvl in range(2, N_LEVELS):
                    S = S0 >> lvl
                    half = S // 2
                    xe2_tiles = []
                    for kind in range(2):
                        cs = X_PLANS[lvl][0]
                        M = half
                        psf = ypsum.tile([128, 512], F32, tag="yps")
                        ps = psf[0:M, 0:S]
                        for i, c in enumerate(cs):
                            ap, K, isbf = cur_tiles[c]
                            if lvl == 2:
                                w_ap = wyb_sb[0:K, 128 * wb2 : 128 * wb2 + M]
                                wb2 += 1
                            else:
                                w_ap = wyf_sb[0:K, 128 * wf34 : 128 * wf34 + M]
                                wf34 += 1
                            nc.tensor.matmul(
                                ps,
                                w_ap,
                                ap,
                                start=(i == 0),
                                stop=(i == len(cs) - 1),
                                skip_group_check=True,
                            )
                        if kind == 0:
                            if lvl == 2:
                                p0, p1, q0, q1 = DEEP_SLOTS["l2xo2"]
                                nc.scalar.copy(stg[p0:p1, q0:q1], ps)
                            else:
                                key = "l3xo2" if lvl == 3 else "l4xo2"
                                p0, p1, q0, q1 = DEEP_SLOTS[key]
                                xo2s = work.tile([M, S], F32, tag=f"xo2s_{lvl}")
                                nc.scalar.copy(xo2s[:], ps)
                                nc.sync.dma_start(stg[p0:p1, q0:q1], xo2s[:])
                        else:
                            xe2 = work.tile([M, S], F32, tag=f"xe2_{lvl}")
                            nc.scalar.copy(xe2[:], ps)
                            xe2_tiles.append((xe2, M))

                    new_cur = []
                    for xe2, P in xe2_tiles:
                        ye_v = xe2[0:P, 0:S:2]
                        yo_v = xe2[0:P, 1:S:2]
                        yo1 = work.tile([P, half], F32, tag=f"yo1_{lvl}")
                        _conv_step(nc, yo1[:], yo_v, ye_v, TP_NPY, tp_sb, P, half)
                        yo2_t = work.tile([P, half], F32, tag=f"yo2_{lvl}")
                        _conv_step(nc, yo2_t[:], yo1[:], ye_v, TP_NCY, tp_sb, P, half)
                        key = {2: "l2yo2", 3: "l3yo2", 4: "l4yo2"}[lvl]
                        p0, p1, q0, q1 = DEEP_SLOTS[key]
                        nc.sync.dma_start(stg[p0:p1, q0:q1], yo2_t[:])
                        ye2 = work.tile([P, half], F32, tag=f"ye2_{lvl}")
                        _conv_step(nc, ye2[:], ye_v, yo2_t[:], TP_RY, tp_sb, P, half)
                        if lvl < N_LEVELS - 1:
                            new_cur.append((ye2[:], P, False))
                        else:
                            p0, p1, q0, q1 = DEEP_SLOTS["ye4"]
                            nc.sync.dma_start(stg[p0:p1, q0:q1], ye2[:])
                    cur_tiles = new_cur

                # ================= binning prep ===========================
                sc = scratch

                # ---- img (from x_flat f32): bin in [128, 256]; h in [16,32)
                bi_i = sc.tile([128, STG], I32, tag="A")
                V("bi_i").tensor_scalar(bi_i[:], x_flat, 128.0, C0, ALU.mult, ALU.add)
                l_ii = sc.tile([128, STG], I32, tag="B")
                V("l_i").tensor_scalar(l_ii[:], bi_i[:], 7, None, ALU.bitwise_and)
                h_ii = sc.tile([128, STG], I32, tag="C")
                V("h_i").tensor_scalar(h_ii[:], bi_i[:], 3, None, ALU.arith_shift_right)
                lb_i = sc.tile([128, STG], BF16, tag="D")
                nc.scalar.activation(lb_i[:], l_ii[:], ACTF.Identity)
                hb_i = sc.tile([128, STG], BF16, tag="E")
                nc.scalar.activation(hb_i[:], h_ii[:], ACTF.Identity)
                junk_i = sc.tile([128, STG], F32, tag="F")
                nc.scalar.activation(
                    junk_i[:], x_flat, ACTF.Square, accum_out=acc[:, s * 8 + 4 : s * 8 + 5]
                )

                ps_if = psum.tile([128, 128], F32, tag="ps")
                ps_i = ps_if[:, 0:64]
                _hist_pipeline(
                    nc, None, pools, hb_i[:], lb_i[:], 16, 16, 8, ps_i,
                    POOL_PLANES_IH,
                )
                pi_sb = work.tile([128, 64], F32, tag="pi_sb")
                nc.scalar.copy(pi_sb[:], ps_i)
                nc.sync.dma_start(pi.ap()[s], pi_sb[:])

                # ---- delta (from stg f32)
                bi_d = sc.tile([128, STG], I32, tag="A")
                V("bi_d").tensor_scalar(bi_d[:], stg[:], 128.0, C0, ALU.mult, ALU.add)
                l_d = sc.tile([128, STG], I32, tag="B")
                V("l_d").tensor_scalar(l_d[:], bi_d[:], 15, None, ALU.bitwise_and)
                h4_d = sc.tile([128, STG], I32, tag="C")
                V("h4_d").tensor_scalar(h4_d[:], bi_d[:], 4, None, ALU.arith_shift_right)
                h_d = sc.tile([128, STG], I32, tag="F")
                V("h_d").tensor_scalar(h_d[:], h4_d[:], 15, None, ALU.bitwise_and)
                lb_d = sc.tile([128, STG], BF16, tag="D")
                nc.scalar.activation(lb_d[:], l_d[:], ACTF.Identity)
                hb_d = sc.tile([128, STG], BF16, tag="E")
                nc.scalar.activation(hb_d[:], h_d[:], ACTF.Identity)
                cng = sc.tile([128, STG], BF16, tag="G")
                V("cng").tensor_scalar(cng[:], stg[:], -1.0, None, ALU.is_lt)
                lx = sc.tile([128, STG], BF16, tag="H")
                nc.vector.scalar_tensor_tensor(
                    lx[:], cng[:], 16.0, lb_d[:], ALU.mult, ALU.add
                )
                fl = sc.tile([128, STG], I32, tag="B")
                V("fl").tensor_scalar(fl[:], bi_d[:], 8, None, ALU.arith_shift_right)
                flf = sc.tile([128, STG], F32, tag="C")
                nc.scalar.activation(flf[:], fl[:], ACTF.Identity, scale=-2.0)
                dl = sc.tile([128, STG], F32, tag="A")
                V("dl").tensor_tensor(dl[:], stg[:], flf[:], ALU.add)
                junk_d = sc.tile([128, STG], F32, tag="B")
                nc.scalar.activation(
                    junk_d[:], dl[:], ACTF.Square, accum_out=acc[:, s * 8 : s * 8 + 1]
                )
                junk_d2 = sc.tile([128, STG], F32, tag="C")
                nc.vector.affine_mul_reduce(
                    junk_d2[:], acc[:, s * 8 + 1 : s * 8 + 2], dl[:], cng[:], -4.0, 4.0
                )

                ps_d = psum.tile([128, 128], F32, tag="ps")
                _hist_pipeline(
                    nc, None, pools, hb_d[:], lx[:], 16, 0, 16, ps_d[:],
                    POOL_PLANES_DH,
                )
                pd_sb = work.tile([128, 128], F32, tag="pd_sb")
                nc.scalar.copy(pd_sb[:], ps_d[:])
                nc.sync.dma_start(pd.ap()[s], pd_sb[:])

            nc.sync.dma_start(accd.ap()[:, :], acc[:])

    nc.compile()
    return nc


_NC_CACHE = {}


def _get_nc():
    if "nc" not in _NC_CACHE:
        _NC_CACHE["nc"] = build_nc()
    return _NC_CACHE["nc"]


LAST_INFO = {}


def kernel(x, px, ux, cx, rx, py, uy, cy, ry, _trace=False):
    x = np.ascontiguousarray(np.asarray(x, dtype=np.float32))
    px, ux, cx, rx, py, uy, cy, ry = (
        np.asarray(k, dtype=np.float32) for k in (px, ux, cx, rx, py, uy, cy, ry)
    )

    nc = _get_nc()

    tp_host = np.zeros(NT, np.float32)
    tp_host[TP_RY : TP_RY + 3] = ry
    tp_host[TP_NPY : TP_NPY + 3] = -py
    tp_host[TP_NCY : TP_NCY + 3] = -cy
    wyf_host, wyb_f = _build_w_host(px, ux, cx, rx, py, uy, cy, ry)
    import ml_dtypes

    wyb_host = wyb_f.astype(ml_dtypes.bfloat16)

    # the device works on W-major (transposed) slices so the x-phase convs
    # run along partitions (TensorE)
    shards = np.ascontiguousarray(
        x.reshape(N_CORES, NSL, S0, S0).transpose(0, 1, 3, 2)
    )
    in_maps = [
        {"xs": np.ascontiguousarray(shards[i]), "tp": tp_host, "wyf": wyf_host,
         "wyb": wyb_host}
        for i in range(N_CORES)
    ]
    if not _trace:
        os.environ.setdefault("BASS_NEVER_TRACE", "1")
    res = run_bass_kernel_spmd(nc, in_maps, core_ids=list(range(N_CORES)), trace=_trace)
    LAST_INFO["exec_time_ns"] = res.exec_time_ns
    LAST_INFO["results"] = res

    counts_img = np.zeros((96, 256))
    counts_delta = np.zeros((96, 256))
    ss_img = np.zeros(96)
    ss_delta = np.zeros(96)
    for core in range(N_CORES):
        out = res.results[core]
        pd_ = out["pd"].astype(np.float64)
        pi_ = out["pi"].astype(np.float64)
        acc_ = out["accd"].astype(np.float64).sum(axis=0)
        for s in range(NSL):
            gs = core * NSL + s
            cd = np.einsum("afbf->ab", pd_[s].reshape(16, 8, 16, 8)).reshape(256)
            ci = np.einsum("afbf->ab", pi_[s].reshape(16, 8, 8, 8)).reshape(128)
            a = acc_[s * 8 : s * 8 + 8]
            counts_delta[gs] = cd
            counts_img[gs, 128:256] = ci
            ss_delta[gs] = a[0] + a[1]
            ss_img[gs] = a[4]

    loss1 = np.float32(255.0 * np.sqrt(ss_delta.sum() / (96 * RES)))
    loss0 = np.float32(255.0 * np.sqrt(ss_img.sum() / (96 * RES)))

    def ent(counts):
        p = counts / RES
        pz = np.where(p > 0, p, 1.0)
        return float(np.sum(-p * np.log2(pz)))

    invCR0 = np.float32(ent(counts_img) / (8.0 * 96))
    invCR1 = np.float32(ent(counts_delta) / (8.0 * 96))
    LAST_INFO.update(
        counts_img=counts_img, counts_delta=counts_delta, ss_img=ss_img, ss_delta=ss_delta
    )
    return loss1, loss0, invCR0, invCR1


# revision 8
# speedup vs baseline: 1.5527x; 1.2088x over previous
"""Trainium2 Bass kernel for nn_Codec (5-level lifting wavelet codec stats).

kernel(**inputs) takes the FULL inputs (x [32,3,512,512] f32 + eight 3-tap
filters) and returns (loss1, loss0, invCR0, invCR1) as np.float32 scalars.

Sharding: pure data parallel - 96 (batch*channel) slices split 12 per core
across 8 NeuronCores; scalar partials are gathered and reduced on the host.

Per-slice device pipeline (v4):
  - Input slices shipped bf16 (host-cast, halves the HBM load).
  - Lifting levels 0-1 entirely on TensorE as bf16 banded matmuls against
    host-composed matrices, with bf16 DMA-xbar transposes between the x and
    y phases. Levels 2-4 (tiny) keep PE x-matmuls + DVE y-convs, all bf16.
  - Subbands land in a [128, 2048] bf16 staging tile (orientation-free for
    the histogram); deep sub-128-partition bands placed by small DMAs.
  - Integer-direct binning: bi = rne(v*128 + 127.5+2^-10) == 128+floor(128v)
    exactly for bf16 v; bin = bi & 255 via shift/mask; invalid (v < -1)
    elements pushed out of the l one-hot range (l += 16).
  - Radix-16x16 (delta) and 16x8 (img) one-hot bf16 mask planes at FC=1024,
    written by DVE (4x mode) with a tunable share on GPSIMD (which supports
    tensor_scalar is_equal); joint counts via TensorE into PSUM.
  - RMSE sums via ScalarE Square+accum (split slots per half-chunk); the
    fmod-vs-posmod correction via affine_mul_reduce.
"""

import os

import numpy as np
from contextlib import ExitStack

import concourse.bass as bass
import concourse.mybir as mybir
import concourse.tile as tile
from concourse import bacc
from concourse.bass_utils import run_bass_kernel_spmd

F32 = mybir.dt.float32
BF16 = mybir.dt.bfloat16
I32 = mybir.dt.int32
ALU = mybir.AluOpType
ACTF = mybir.ActivationFunctionType

N_CORES = 8
S0 = 512
NSL = 12            # slices per core (96 / 8)
STG = 2048          # staging free dim per slice (512*512/128)
RES = S0 * S0
FC = 1024           # mask chunk width (free dim)
HC = 1024           # prep sub-chunk width
N_LEVELS = 5
G = 8               # sub-chunk dup factor for the joint-count matmuls
C0 = 127.5009765625  # 127.5 + 2^-10: rne(v*128 + C0) == 128 + floor(v*128)

# tap vector layout (y-phase DVE convs for levels 2-4 only).
TP_RY, TP_NPY, TP_NCY = 0, 3, 6
NT = 9

# ---------------------------------------------------------------------------
# host-side matrix composition + block plans
# ---------------------------------------------------------------------------


def _make_mats(S, p, u, c, r, update):
    """A (odd out) and B (even out) lifting matrices [S/2, S], f64->f32.
    update=False omits the update step (y-lifting at levels >= 2)."""
    half = S // 2
    E = np.zeros((half, S))
    O = np.zeros((half, S))
    E[np.arange(half), 2 * np.arange(half)] = 1.0
    O[np.arange(half), 2 * np.arange(half) + 1] = 1.0

    def T(k):
        M = np.zeros((half, half))
        i = np.arange(half)
        M[i, i] = k[1]
        M[i[1:], i[1:] - 1] = k[0]
        M[i[:-1], i[:-1] + 1] = k[2]
        return M

    o1 = O - T(p.astype(np.float64)) @ E
    e1 = E + T(u.astype(np.float64)) @ o1 if update else E
    A = o1 - T(c.astype(np.float64)) @ e1
    B = e1 + T(r.astype(np.float64)) @ A
    return A.astype(np.float32), B.astype(np.float32)


def _plan(S):
    """Static nonzero-block structure for a [S/2, S] composed lifting matrix
    (band halfwidth <= 10 in the S domain): per out-tile r, the list of
    128-wide K-block cols that are structurally nonzero."""
    half = S // 2
    t_out = max(1, (half + 127) // 128)
    t_in = max(1, (S + 127) // 128)
    rows = []
    for r in range(t_out):
        m0 = 128 * r
        m1 = min(m0 + 128, half)
        j0 = max(0, 2 * m0 - 10)
        j1 = min(S - 1, 2 * (m1 - 1) + 10)
        rows.append([c for c in range(t_in) if 128 * c <= j1 and 128 * c + 127 >= j0])
    return rows


X_PLANS = [_plan(S0 >> lvl) for lvl in range(N_LEVELS)]
Y_PLANS = [_plan(S0 >> lvl) for lvl in range(2)]  # y on PE for lvl 0-1 only

# wyb (bf16) block order: x lvl0 A/B, y lvl0 C/R, x lvl1 A/B, y lvl1 C/R,
# x lvl2 A/B, x lvl3 A/B, x lvl4 A/B
NWB = (
    2 * sum(len(cs) for cs in X_PLANS[0])
    + 2 * sum(len(cs) for cs in Y_PLANS[0])
    + 2 * sum(len(cs) for cs in X_PLANS[1])
    + 2 * sum(len(cs) for cs in Y_PLANS[1])
    + 2 * sum(len(cs) for cs in X_PLANS[2])
    + 2 * sum(len(cs) for cs in X_PLANS[3])
    + 2 * sum(len(cs) for cs in X_PLANS[4])
)


def _pack_blocks(M_, plan, buf, i):
    half, S = M_.shape[0], M_.shape[1]
    for r, cs in enumerate(plan):
        m0, m1 = 128 * r, min(128 * r + 128, half)
        for c in cs:
            k0, k1 = 128 * c, min(128 * c + 128, S)
            buf[i, : k1 - k0, : m1 - m0] = M_[m0:m1, k0:k1].T
            i += 1
    return i


def _check_cover(M_, plan):
    half, S = M_.shape
    mass = np.abs(M_).sum()
    cov = 0.0
    for r, cs in enumerate(plan):
        m0, m1 = 128 * r, min(128 * r + 128, half)
        for c in cs:
            k0, k1 = 128 * c, min(128 * c + 128, S)
            cov += np.abs(M_[m0:m1, k0:k1]).sum()
    assert abs(cov - mass) < 1e-6 * max(mass, 1), (half, S, cov, mass)


def _build_w_host(px, ux, cx, rx, py, uy, cy, ry):
    xmats = [_make_mats(S0 >> l, px, ux, cx, rx, update=True) for l in range(N_LEVELS)]
    ymats = [_make_mats(S0 >> l, py, uy, cy, ry, update=(l < 2)) for l in range(2)]
    for lvl in range(N_LEVELS):
        for M_ in xmats[lvl]:
            _check_cover(M_, X_PLANS[lvl])
    for lvl in range(2):
        for M_ in ymats[lvl]:
            _check_cover(M_, Y_PLANS[lvl])

    wyb = np.zeros((NWB, 128, 128), np.float32)
    i = 0
    for M_ in xmats[0]:
        i = _pack_blocks(M_, X_PLANS[0], wyb, i)
    for M_ in ymats[0]:
        i = _pack_blocks(M_, Y_PLANS[0], wyb, i)
    for M_ in xmats[1]:
        i = _pack_blocks(M_, X_PLANS[1], wyb, i)
    for M_ in ymats[1]:
        i = _pack_blocks(M_, Y_PLANS[1], wyb, i)
    for lvl in (2, 3, 4):
        for M_ in xmats[lvl]:
            i = _pack_blocks(M_, X_PLANS[lvl], wyb, i)
    assert i == NWB, (i, NWB)
    return wyb


# staging slots for the deep subbands (levels 2-4). (p0, p1, c0, c1)
DEEP_SLOTS = {
    "l2xo2": (0, 64, 1920, 2048),
    "l2yo2": (64, 128, 1920, 1984),
    "l3xo2": (64, 96, 1984, 2048),
    "l3yo2": (96, 128, 1984, 2016),
    "l4xo2": (96, 112, 2016, 2048),
    "l4yo2": (112, 128, 2016, 2032),
    "ye4": (112, 128, 2032, 2048),
}

# elementwise prep routing: "v" = DVE, "g" = GPSIMD (tensor_scalar-compatible
# ops only: converts/compares; int bit-ops and STT must stay on DVE)
ROUTE = {"bi_d": "g", "bi_i": "g", "cng": "g"}
# per-chunk count of h one-hot planes routed to GPSIMD
POOL_PLANES_DH = 5
POOL_PLANES_IH = 5

# ---------------------------------------------------------------------------
# device kernel
# ---------------------------------------------------------------------------


def _conv_step(nc, out_ap, base_ap, src_ap, tap_col, tp_sb, P, F):
    """out = base + 3-tap conv of src along the free dim, zero padding."""
    k0 = tp_sb[0:P, tap_col : tap_col + 1]
    k1 = tp_sb[0:P, tap_col + 1 : tap_col + 2]
    k2 = tp_sb[0:P, tap_col + 2 : tap_col + 3]
    nc.vector.scalar_tensor_tensor(out_ap, src_ap, k1, base_ap, ALU.mult, ALU.add)
    nc.vector.scalar_tensor_tensor(
        out_ap[:, 1:F], src_ap[:, 0 : F - 1], k0, out_ap[:, 1:F], ALU.mult, ALU.add
    )
    nc.vector.scalar_tensor_tensor(
        out_ap[:, 0 : F - 1], src_ap[:, 1:F], k2, out_ap[:, 0 : F - 1], ALU.mult, ALU.add
    )


def _hist_pipeline(nc, pools, hb_src, lb_src, n_h, h0, n_l, psum_ap, pool_h):
    """One-hot mask + joint-count matmuls over [128, STG] bf16 digit sources."""
    mpool = pools["masks"]
    n_chunks = STG // FC
    n_mm = FC // G
    for ch in range(n_chunks):
        c0 = ch * FC
        mh = mpool.tile([128, n_mm, 16 * G], BF16, tag="mh")
        ml = mpool.tile([128, n_mm, 16 * G], BF16, tag="ml")
        for a in range(n_h):
            e = nc.gpsimd if a < pool_h else nc.vector
            e.tensor_scalar(
                mh[:, :, a * G : (a + 1) * G],
                hb_src[:, c0 : c0 + FC],
                float(h0 + a),
                None,
                ALU.is_equal,
            )
        for b in range(n_l):
            nc.vector.tensor_scalar(
                ml[:, :, b * G : (b + 1) * G],
                lb_src[:, c0 : c0 + FC],
                float(b),
                None,
                ALU.is_equal,
            )
        for g_ in range(n_mm):
            nc.tensor.matmul(
                psum_ap,
                mh[:, g_, 0 : n_h * G],
                ml[:, g_, 0 : n_l * G],
                start=(ch == 0 and g_ == 0),
                stop=(ch == n_chunks - 1 and g_ == n_mm - 1),
                skip_group_check=True,
            )


def build_nc(nsl=NSL):
    nc = bacc.Bacc("TRN2", target_bir_lowering=False, debug=False)
    xs = nc.dram_tensor("xs", [nsl, S0, S0], BF16, kind="ExternalInput")
    tp = nc.dram_tensor("tp", [NT], F32, kind="ExternalInput")
    wyb = nc.dram_tensor("wyb", [NWB, 128, 128], BF16, kind="ExternalInput")
    pd = nc.dram_tensor("pd", [nsl, 128, 128], F32, kind="ExternalOutput")
    pi = nc.dram_tensor("pi", [nsl, 128, 64], F32, kind="ExternalOutput")
    accd = nc.dram_tensor("accd", [128, nsl * 8], F32, kind="ExternalOutput")

    def V(name):
        return nc.gpsimd if ROUTE.get(name) == "g" else nc.vector

    with tile.TileContext(nc) as tc:
        with ExitStack() as ctx:
            const = ctx.enter_context(tc.tile_pool(name="const", bufs=1))
            xpool = ctx.enter_context(tc.tile_pool(name="xpool", bufs=2))
            stgp = ctx.enter_context(tc.tile_pool(name="stgp", bufs=2))
            lift = ctx.enter_context(tc.tile_pool(name="lift", bufs=2))
            work = ctx.enter_context(tc.tile_pool(name="work", bufs=3))
            scratch = ctx.enter_context(tc.tile_pool(name="scratch", bufs=1))
            maskp = ctx.enter_context(tc.tile_pool(name="masks", bufs=2))
            psum = ctx.enter_context(tc.tile_pool(name="psum", bufs=2, space="PSUM"))
            ypsum = ctx.enter_context(tc.tile_pool(name="ypsum", bufs=2, space="PSUM"))

            tp_sb = const.tile([128, NT], F32)
            nc.sync.dma_start(
                tp_sb[:], tp.ap().rearrange("(o n) -> o n", o=1).broadcast_to([128, NT])
            )
            wyb_sb = const.tile([128, NWB * 128], BF16)
            nc.sync.dma_start(
                wyb_sb[:].rearrange("k (n m) -> k n m", n=NWB),
                wyb.ap().rearrange("n k m -> k n m"),
            )
            acc = const.tile([128, nsl * 8], F32)
            nc.vector.memset(acc[:], 0.0)
            pools = {"masks": maskp}

            nb_x0 = 2 * sum(len(cs) for cs in X_PLANS[0])
            nb_y0 = nb_x0 + 2 * sum(len(cs) for cs in Y_PLANS[0])
            nb_x1 = nb_y0 + 2 * sum(len(cs) for cs in X_PLANS[1])
            nb_y1 = nb_x1 + 2 * sum(len(cs) for cs in Y_PLANS[1])

            for s in range(nsl):
                # ---- load slice (transposed, bf16): x_sb[p, t, h]
                x_sb = xpool.tile([128, 4, S0], BF16, tag="x_sb")
                nc.sync.dma_start(
                    x_sb[:], xs.ap()[s].rearrange("(t p) w -> p t w", p=128)
                )
                x_flat = x_sb[:].rearrange("p t w -> p (t w)")

                stg = stgp.tile([128, STG], BF16, tag="stg")

                # ================= level 0 (PE, bf16) =====================
                xe2b = lift.tile([128, 2, S0], BF16, tag="xe2b0")
                wf = 0
                for kind in range(2):  # 0: A (xo), 1: B (xe2)
                    for r, cs in enumerate(X_PLANS[0]):
                        ps = ypsum.tile([128, 512], F32, tag="yps")
                        for i, c in enumerate(cs):
                            nc.tensor.matmul(
                                ps[:, :],
                                wyb_sb[0:128, 128 * wf : 128 * wf + 128],
                                x_sb[:, c, :],
                                start=(i == 0),
                                stop=(i == len(cs) - 1),
                                skip_group_check=True,
                            )
                            wf += 1
                        if kind == 0:
                            nc.scalar.copy(stg[:, 512 * r : 512 * (r + 1)], ps[:, :])
                        else:
                            nc.scalar.copy(xe2b[:, r, :], ps[:, :])
                assert wf == nb_x0

                # transpose xe2 [256, 512] -> xe2T (bf16 xbar)
                xe2T = lift.tile([128, 2, 4, 128], BF16, tag="xe2T0")
                for r in range(2):
                    nc.sync.dma_start_transpose(xe2T[:, r, :, :], xe2b[:, r, :])

                # y-phase: yo = C0 @ xe2T, ye2 = R0 @ xe2T, per (r2, r)
                ye2b = lift.tile([128, 2, 2, 128], BF16, tag="ye2b0")
                wb = nb_x0
                for kind in range(2):  # 0: C (yo), 1: R (ye2)
                    for r2, cs in enumerate(Y_PLANS[0]):
                        blk0 = wb
                        for r in range(2):
                            ps2f = ypsum.tile([128, 512], F32, tag="yps")
                            ps2 = ps2f[:, 0:128]
                            for i, c2 in enumerate(cs):
                                nc.tensor.matmul(
                                    ps2,
                                    wyb_sb[0:128, 128 * (blk0 + i) : 128 * (blk0 + i) + 128],
                                    xe2T[:, r, c2, :],
                                    start=(i == 0),
                                    stop=(i == len(cs) - 1),
                                    skip_group_check=True,
                                )
                            if kind == 0:
                                col = 1024 + 128 * (2 * r2 + r)
                                nc.scalar.copy(stg[:, col : col + 128], ps2)
                            else:
                                nc.scalar.copy(ye2b[:, r2, r, :], ps2)
                        wb = blk0 + len(cs)
                assert wb == nb_y0

                cur1 = lift.tile([128, 2, 256], BF16, tag="cur1")
                for r2 in range(2):
                    for r in range(2):
                        nc.sync.dma_start_transpose(
                            cur1[:, r, 128 * r2 : 128 * r2 + 128], ye2b[:, r2, r, :]
                        )

                # ================= level 1 (PE bf16) ======================
                xe2b1 = lift.tile([128, 256], BF16, tag="xe2b1")
                for kind in range(2):
                    base = nb_y0 + (0 if kind == 0 else len(X_PLANS[1][0]))
                    psf = ypsum.tile([128, 512], F32, tag="yps")
                    ps = psf[:, 0:256]
                    cs = X_PLANS[1][0]
                    for i, c in enumerate(cs):
                        nc.tensor.matmul(
                            ps,
                            wyb_sb[0:128, 128 * (base + i) : 128 * (base + i) + 128],
                            cur1[:, c, :],
                            start=(i == 0),
                            stop=(i == len(cs) - 1),
                            skip_group_check=True,
                        )
                    if kind == 0:
                        nc.scalar.copy(stg[:, 1536:1792], ps)
                    else:
                        nc.scalar.copy(xe2b1[:], ps)

                xe2T1 = lift.tile([128, 2, 128], BF16, tag="xe2T1")
                nc.sync.dma_start_transpose(xe2T1[:, :, :], xe2b1[:])

                ye2b1 = lift.tile([128, 128], BF16, tag="ye2b1")
                for kind in range(2):
                    base = nb_x1 + (0 if kind == 0 else len(Y_PLANS[1][0]))
                    ps2f = ypsum.tile([128, 512], F32, tag="yps")
                    ps2 = ps2f[:, 0:128]
                    cs = Y_PLANS[1][0]
                    for i, c2 in enumerate(cs):
                        nc.tensor.matmul(
                            ps2,
                            wyb_sb[0:128, 128 * (base + i) : 128 * (base + i) + 128],
                            xe2T1[:, c2, :],
                            start=(i == 0),
                            stop=(i == len(cs) - 1),
                            skip_group_check=True,
                        )
                    if kind == 0:
                        nc.scalar.copy(stg[:, 1792:1920], ps2)
                    else:
                        nc.scalar.copy(ye2b1[:], ps2)

                cur2 = lift.tile([128, 128], BF16, tag="cur2")
                nc.sync.dma_start_transpose(cur2[:, :], ye2b1[:])

                # ================= levels 2-4 (PE x bf16, DVE y) ==========
                cur_tiles = [(cur2[:, :], 128)]
                wb2 = nb_y1
                for lvl in range(2, N_LEVELS):
                    S = S0 >> lvl
                    half = S // 2
                    xe2_tiles = []
                    for kind in range(2):
                        cs = X_PLANS[lvl][0]
                        M = half
                        psf = ypsum.tile([128, 512], F32, tag="yps")
                        ps = psf[0:M, 0:S]
                        for i, c in enumerate(cs):
                            ap, K = cur_tiles[c]
                            nc.tensor.matmul(
                                ps,
                                wyb_sb[0:K, 128 * wb2 : 128 * wb2 + M],
                                ap,
                                start=(i == 0),
                                stop=(i == len(cs) - 1),
                                skip_group_check=True,
                            )
                            wb2 += 1
                        if kind == 0:
                            if lvl == 2:
                                p0, p1, q0, q1 = DEEP_SLOTS["l2xo2"]
                                nc.scalar.copy(stg[p0:p1, q0:q1], ps)
                            else:
                                key = "l3xo2" if lvl == 3 else "l4xo2"
                                p0, p1, q0, q1 = DEEP_SLOTS[key]
                                xo2s = work.tile([M, S], BF16, tag=f"xo2s_{lvl}")
                                nc.scalar.copy(xo2s[:], ps)
                                nc.sync.dma_start(stg[p0:p1, q0:q1], xo2s[:])
                        else:
                            xe2 = work.tile([M, S], BF16, tag=f"xe2_{lvl}")
                            nc.scalar.copy(xe2[:], ps)
                            xe2_tiles.append((xe2, M))

                    new_cur = []
                    for xe2, P in xe2_tiles:
                        ye_v = xe2[0:P, 0:S:2]
                        yo_v = xe2[0:P, 1:S:2]
                        yo1 = work.tile([P, half], BF16, tag=f"yo1_{lvl}")
                        _conv_step(nc, yo1[:], yo_v, ye_v, TP_NPY, tp_sb, P, half)
                        yo2_t = work.tile([P, half], BF16, tag=f"yo2_{lvl}")
                        _conv_step(nc, yo2_t[:], yo1[:], ye_v, TP_NCY, tp_sb, P, half)
                        key = {2: "l2yo2", 3: "l3yo2", 4: "l4yo2"}[lvl]
                        p0, p1, q0, q1 = DEEP_SLOTS[key]
                        nc.sync.dma_start(stg[p0:p1, q0:q1], yo2_t[:])
                        ye2 = work.tile([P, half], BF16, tag=f"ye2_{lvl}")
                        _conv_step(nc, ye2[:], ye_v, yo2_t[:], TP_RY, tp_sb, P, half)
                        if lvl < N_LEVELS - 1:
                            new_cur.append((ye2[:], P))
                        else:
                            p0, p1, q0, q1 = DEEP_SLOTS["ye4"]
                            nc.sync.dma_start(stg[p0:p1, q0:q1], ye2[:])
                    cur_tiles = new_cur

                # ================= binning prep ===========================
                sc = scratch
                NH = STG // HC  # prep sub-chunks

                # ---- img (x bf16): bin = bi in [128, 256]; h=bi>>3, l=bi&7
                lb_i = sc.tile([128, STG], BF16, tag="lb_i")
                hb_i = sc.tile([128, STG], BF16, tag="hb_i")
                for h in range(NH):
                    lo, hi = h * HC, (h + 1) * HC
                    bi_i = sc.tile([128, HC], I32, tag="A")
                    V("bi_i").tensor_scalar(
                        bi_i[:], x_flat[:, lo:hi], 128.0, C0, ALU.mult, ALU.add
                    )
                    l_ii = sc.tile([128, HC], I32, tag="B")
                    nc.vector.tensor_scalar(l_ii[:], bi_i[:], 7, None, ALU.bitwise_and)
                    h_ii = sc.tile([128, HC], I32, tag="C")
                    nc.vector.tensor_scalar(
                        h_ii[:], bi_i[:], 3, None, ALU.arith_shift_right
                    )
                    nc.scalar.activation(lb_i[:, lo:hi], l_ii[:], ACTF.Identity)
                    nc.scalar.activation(hb_i[:, lo:hi], h_ii[:], ACTF.Identity)
                    junk_i = sc.tile([128, HC], F32, tag="A")
                    nc.scalar.activation(
                        junk_i[:], x_flat[:, lo:hi], ACTF.Square,
                        accum_out=acc[:, s * 8 + 4 + h : s * 8 + 5 + h],
                    )

                ps_if = psum.tile([128, 128], F32, tag="ps")
                ps_i = ps_if[:, 0:64]
                _hist_pipeline(
                    nc, pools, hb_i[:], lb_i[:], 16, 16, 8, ps_i, POOL_PLANES_IH
                )
                pi_sb = work.tile([128, 64], F32, tag="pi_sb")
                nc.scalar.copy(pi_sb[:], ps_i)
                nc.sync.dma_start(pi.ap()[s], pi_sb[:])

                # ---- delta (stg bf16)
                lx = sc.tile([128, STG], BF16, tag="lx")
                hb_d = sc.tile([128, STG], BF16, tag="hb_d")
                for h in range(NH):
                    lo, hi = h * HC, (h + 1) * HC
                    bi_d = sc.tile([128, HC], I32, tag="A")
                    V("bi_d").tensor_scalar(
                        bi_d[:], stg[:, lo:hi], 128.0, C0, ALU.mult, ALU.add
                    )
                    l_d = sc.tile([128, HC], I32, tag="B")
                    nc.vector.tensor_scalar(l_d[:], bi_d[:], 15, None, ALU.bitwise_and)
                    h_d = sc.tile([128, HC], I32, tag="C")
                    nc.vector.tensor_scalar(
                        h_d[:], bi_d[:], 4, 15, ALU.arith_shift_right, ALU.bitwise_and
                    )
                    lb_d = sc.tile([128, HC], BF16, tag="D")
                    nc.scalar.activation(lb_d[:], l_d[:], ACTF.Identity)
                    nc.scalar.activation(hb_d[:, lo:hi], h_d[:], ACTF.Identity)
                    cng = sc.tile([128, HC], BF16, tag="E")
                    V("cng").tensor_scalar(cng[:], stg[:, lo:hi], -1.0, None, ALU.is_lt)
                    nc.vector.scalar_tensor_tensor(
                        lx[:, lo:hi], cng[:], 16.0, lb_d[:], ALU.mult, ALU.add
                    )
                    fl = sc.tile([128, HC], I32, tag="B")
                    nc.vector.tensor_scalar(
                        fl[:], bi_d[:], 8, None, ALU.arith_shift_right
                    )
                    dl = sc.tile([128, HC], F32, tag="C")
                    nc.vector.scalar_tensor_tensor(
                        dl[:], fl[:], -2.0, stg[:, lo:hi], ALU.mult, ALU.add
                    )
                    junk_d = sc.tile([128, HC], F32, tag="A")
                    nc.scalar.activation(
                        junk_d[:], dl[:], ACTF.Square,
                        accum_out=acc[:, s * 8 + h : s * 8 + 1 + h],
                    )
                    junk_d2 = sc.tile([128, HC], F32, tag="B")
                    nc.vector.affine_mul_reduce(
                        junk_d2[:], acc[:, s * 8 + 2 + h : s * 8 + 3 + h],
                        dl[:], cng[:], -4.0, 4.0,
                    )

                ps_d = psum.tile([128, 128], F32, tag="ps")
                _hist_pipeline(
                    nc, pools, hb_d[:], lx[:], 16, 0, 16, ps_d[:], POOL_PLANES_DH
                )
                pd_sb = work.tile([128, 128], F32, tag="pd_sb")
                nc.scalar.copy(pd_sb[:], ps_d[:])
                nc.sync.dma_start(pd.ap()[s], pd_sb[:])

            nc.sync.dma_start(accd.ap()[:, :], acc[:])

    nc.compile()
    return nc


_NC_CACHE = {}


def _get_nc():
    if "nc" not in _NC_CACHE:
        _NC_CACHE["nc"] = build_nc()
    return _NC_CACHE["nc"]


LAST_INFO = {}


def kernel(x, px, ux, cx, rx, py, uy, cy, ry, _trace=False):
    import ml_dtypes

    x = np.asarray(x, dtype=np.float32)
    px, ux, cx, rx, py, uy, cy, ry = (
        np.asarray(k, dtype=np.float32) for k in (px, ux, cx, rx, py, uy, cy, ry)
    )

    nc = _get_nc()

    tp_host = np.zeros(NT, np.float32)
    tp_host[TP_RY : TP_RY + 3] = ry
    tp_host[TP_NPY : TP_NPY + 3] = -py
    tp_host[TP_NCY : TP_NCY + 3] = -cy
    wyb_host = _build_w_host(px, ux, cx, rx, py, uy, cy, ry).astype(ml_dtypes.bfloat16)

    # W-major (transposed) slices, cast bf16 on the host
    shards = np.ascontiguousarray(
        x.reshape(N_CORES, NSL, S0, S0).transpose(0, 1, 3, 2)
    ).astype(ml_dtypes.bfloat16)
    in_maps = [
        {"xs": np.ascontiguousarray(shards[i]), "tp": tp_host, "wyb": wyb_host}
        for i in range(N_CORES)
    ]
    if not _trace:
        os.environ.setdefault("BASS_NEVER_TRACE", "1")
    res = run_bass_kernel_spmd(nc, in_maps, core_ids=list(range(N_CORES)), trace=_trace)
    LAST_INFO["exec_time_ns"] = res.exec_time_ns
    LAST_INFO["results"] = res

    counts_img = np.zeros((96, 256))
    counts_delta = np.zeros((96, 256))
    ss_img = np.zeros(96)
    ss_delta = np.zeros(96)
    for core in range(N_CORES):
        out = res.results[core]
        pd_ = out["pd"].astype(np.float64)
        pi_ = out["pi"].astype(np.float64)
        acc_ = out["accd"].astype(np.float64).sum(axis=0)
        for s in range(NSL):
            gs = core * NSL + s
            cd = np.einsum("afbf->ab", pd_[s].reshape(16, 8, 16, 8)).reshape(256)
            ci = np.einsum("afbf->ab", pi_[s].reshape(16, 8, 8, 8)).reshape(128)
            a = acc_[s * 8 : s * 8 + 8]
            counts_delta[gs] = cd
            counts_img[gs, 128:256] = ci
            ss_delta[gs] = a[0] + a[1] + a[2] + a[3]
            ss_img[gs] = a[4] + a[5]

    loss1 = np.float32(255.0 * np.sqrt(ss_delta.sum() / (96 * RES)))
    loss0 = np.float32(255.0 * np.sqrt(ss_img.sum() / (96 * RES)))

    def ent(counts):
        p = counts / RES
        pz = np.where(p > 0, p, 1.0)
        return float(np.sum(-p * np.log2(pz)))

    invCR0 = np.float32(ent(counts_img) / (8.0 * 96))
    invCR1 = np.float32(ent(counts_delta) / (8.0 * 96))
    LAST_INFO.update(
        counts_img=counts_img, counts_delta=counts_delta, ss_img=ss_img, ss_delta=ss_delta
    )
    return loss1, loss0, invCR0, invCR1
